# revision 7
# baseline (speedup 1.0000x reference)
"""DecoderTreeLSTM Trainium2 kernel (8 NeuronCores, data-parallel over nodes).

Strategy (sharding_hint: data parallel over the 512 trees):
  Each level has 512 nodes; core k owns nodes [64k, 64k+64) of every level.
  Per level, each core computes its 64 nodes' LSTM cell + output head, then an
  AllGather publishes (h, c, argmax, dist) rows of the level into a Shared
  DRAM STATE tensor replicated on every core.  Parent lookups (arbitrary
  row indices) are indirect-DMA row gathers from STATE.

Key algebraic rewrites (all exact):
  - dropout folded into weights: state stores pre-dropout h2;
    Wh and out_W rows are pre-scaled by dropout_mask on the host.
  - embedding contribution: embed_table[eidx] @ Wx_bot + b
      == onehot(eidx) @ (embed_table @ Wx_bot + b[None,:])  (T2b, on-device)
  - features @ Wx_top (72% of FLOPs) has no recurrence dependency; it is
    computed 2 levels ahead so the PE crunches it during AllGather waits.
"""

import sys
import numpy as np

sys.path.insert(0, "/opt/trn_rl_repo")

R = 8          # cores
LW = 512       # nodes per level
NB = LW // R   # 64 nodes per core per level
DEPTH = 16
N = LW * DEPTH
DIN = 512
H = 512
E = 200
C = 150
CL = C + 1     # embedding rows
G4 = 4 * H     # 2048 gate cols ([i | f | g | o])
NC_ = 4        # 512-wide N-chunks of the gate matmul
SC = H + H + 1 + C       # 1175 state cols: h, c, cm, dist
HCC = H + H + 1          # 1025 gather cols (h, c, cm)
OPC = (N - LW) // R  # 960 output rows per core
FCH = 8                  # final-phase chunks
FRW = OPC // FCH         # 120 rows per final chunk

_CACHE = {}


def _build(reps=1):
    import concourse.bass as bass
    import concourse.mybir as mybir
    import concourse.tile as tile
    from concourse import bacc

    f32 = mybir.dt.float32
    i32 = mybir.dt.int32
    u32 = mybir.dt.uint32
    AF = mybir.ActivationFunctionType
    OP = mybir.AluOpType

    nc = bacc.Bacc("TRN2", target_bir_lowering=False, debug=False, num_devices=R)

    # ---- I/O ----------------------------------------------------------------
    feat_my = nc.dram_tensor("feat_my", [DEPTH * NB, DIN], f32, kind="ExternalInput")
    pp_my = nc.dram_tensor("pp_my", [DEPTH * NB, 1], i32, kind="ExternalInput")
    ord_my = nc.dram_tensor("ord_my", [OPC, 1], i32, kind="ExternalInput")
    WhS_in = nc.dram_tensor("WhS", [H, G4], f32, kind="ExternalInput")
    WxT_in = nc.dram_tensor("WxT", [DIN, G4], f32, kind="ExternalInput")
    WxB_in = nc.dram_tensor("WxB", [E, G4], f32, kind="ExternalInput")
    embT_in = nc.dram_tensor("embT", [E, CL], f32, kind="ExternalInput")
    bRow_in = nc.dram_tensor("bRow", [1, G4], f32, kind="ExternalInput")
    oWs_in = nc.dram_tensor("oWs", [H, C], f32, kind="ExternalInput")
    oB_in = nc.dram_tensor("oB", [1, C], f32, kind="ExternalInput")
    iota_in = nc.dram_tensor("iotaM", [128, CL], f32, kind="ExternalInput")
    ident_in = nc.dram_tensor("ident", [128, 128], f32, kind="ExternalInput")
    ones_in = nc.dram_tensor("ones", [1, 128], f32, kind="ExternalInput")

    dist_out = nc.dram_tensor("dist_out", [OPC, C], f32, kind="ExternalOutput")
    cm_out = nc.dram_tensor("cm_out", [OPC, 1], i32, kind="ExternalOutput")

    state = nc.dram_tensor("state", [N, SC], f32, addr_space="Shared")

    HT = H // 128   # 4 hidden 128-tiles

    with tile.TileContext(nc) as tc:
        with (
            tc.tile_pool(name="wpool", bufs=1) as wp,      # resident weights
            tc.tile_pool(name="work", bufs=1) as sb,       # misc single tiles
            tc.tile_pool(name="ring", bufs=1) as rg,       # ring-buffered tiles
            tc.tile_pool(name="pg", bufs=4, space="PSUM") as pg,
            tc.tile_pool(name="px", bufs=1, space="PSUM") as px,
            tc.tile_pool(name="ps", bufs=3, space="PSUM") as ps,
            tc.tile_pool(name="dram", bufs=2, space="DRAM") as dr,
        ):
            # ---- resident constants / weights -------------------------------
            WhS = [wp.tile([128, G4], f32, name=f"WhS{k}") for k in range(HT)]
            for k in range(HT):
                nc.sync.dma_start(out=WhS[k][:], in_=WhS_in[128 * k : 128 * (k + 1), :])
            WxT = [wp.tile([128, G4], f32, name=f"WxT{k}") for k in range(HT)]
            for k in range(HT):
                nc.sync.dma_start(out=WxT[k][:], in_=WxT_in[128 * k : 128 * (k + 1), :])
            WxB0 = wp.tile([128, G4], f32)
            nc.sync.dma_start(out=WxB0[:], in_=WxB_in[0:128, :])
            WxB1 = wp.tile([E - 128, G4], f32)
            nc.sync.dma_start(out=WxB1[:], in_=WxB_in[128:E, :])
            embT0 = wp.tile([128, CL], f32)
            nc.sync.dma_start(out=embT0[:], in_=embT_in[0:128, :])
            embT1 = wp.tile([E - 128, CL], f32)
            nc.sync.dma_start(out=embT1[:], in_=embT_in[128:E, :])
            bRow = wp.tile([1, G4], f32)
            nc.sync.dma_start(out=bRow[:], in_=bRow_in[:, :])
            oWs = [wp.tile([128, C], f32, name=f"oWs{k}") for k in range(HT)]
            for k in range(HT):
                nc.sync.dma_start(out=oWs[k][:], in_=oWs_in[128 * k : 128 * (k + 1), :])
            oB = wp.tile([1, C], f32)
            nc.sync.dma_start(out=oB[:], in_=oB_in[:, :])
            iotaM = wp.tile([128, CL], f32)
            nc.sync.dma_start(out=iotaM[:], in_=iota_in[:, :])
            ident = wp.tile([128, 128], f32)
            nc.sync.dma_start(out=ident[:], in_=ident_in[:, :])
            ones = wp.tile([1, 128], f32)
            nc.sync.dma_start(out=ones[:], in_=ones_in[:, :])

            # ---- T2b = embed_table @ Wx_bot + b  (shape [CL, G4]) -----------
            T2b0 = wp.tile([128, G4], f32)
            T2b1 = wp.tile([CL - 128, G4], f32)
            for (m0, msz, dst) in ((0, 128, T2b0), (128, CL - 128, T2b1)):
                for n in range(NC_):
                    nsl = slice(512 * n, 512 * (n + 1))
                    pt = ps.tile([128, 512], f32, tag="ps", name=f"t2_{m0}_{n}")
                    nc.tensor.matmul(
                        pt[0:msz, :], lhsT=embT0[:, m0 : m0 + msz],
                        rhs=WxB0[:, nsl], start=True, stop=False)
                    nc.tensor.matmul(
                        pt[0:msz, :], lhsT=embT1[:, m0 : m0 + msz],
                        rhs=WxB1[:, nsl], start=False, stop=False)
                    nc.tensor.matmul(
                        pt[0:msz, :], lhsT=ones[0:1, 0:msz],
                        rhs=bRow[0:1, nsl], start=False, stop=True)
                    nc.scalar.copy(out=dst[:, nsl], in_=pt[0:msz, :])

            # ---- per-level pieces -------------------------------------------
            gx_sb = {}

            def gx_prep(l):
                """g_x[l] = features[level l rows] @ Wx_top   (no recurrence dep)."""
                feat = rg.tile([NB, DIN], f32, tag="feat", bufs=3, name=f"feat{l}")
                nc.sync.dma_start(out=feat[:], in_=feat_my[NB * l : NB * (l + 1), :])
                fT = rg.tile([128, NB * HT], f32, tag="fT", bufs=2, name=f"fT{l}")
                for j in range(HT):
                    ptr = ps.tile([128, NB], f32, tag="ps", name=f"ftr{l}_{j}")
                    nc.tensor.transpose(
                        out=ptr[:], in_=feat[:, 128 * j : 128 * (j + 1)],
                        identity=ident[0:NB, 0:NB])
                    nc.scalar.copy(out=fT[:, NB * j : NB * (j + 1)], in_=ptr[:])
                gx = rg.tile([NB, G4], f32, tag="gx", bufs=3, name=f"gx{l}")
                for n in range(NC_):
                    nsl = slice(512 * n, 512 * (n + 1))
                    pxn = px.tile([NB, 512], f32, tag="px", name=f"gxp{l}_{n}")
                    for k in range(HT):
                        nc.tensor.matmul(
                            pxn[:, :], lhsT=fT[:, NB * k : NB * (k + 1)],
                            rhs=WxT[k][:, nsl], start=(k == 0), stop=(k == HT - 1))
                    nc.scalar.copy(out=gx[:, nsl], in_=pxn[:, :])
                gx_sb[l] = gx

            def recurrence(l):
                # Open all four gate-chunk PSUM accumulations with the
                # recurrence-independent g_x term so the PE has ready work
                # while the previous level's AllGather is still in flight.
                gx = gx_sb.pop(l)
                pgn = []
                for n in range(NC_):
                    nsl = slice(512 * n, 512 * (n + 1))
                    pt = pg.tile([NB, 512], f32, tag="pg", name=f"g{l}_{n}")
                    nc.tensor.matmul(pt[:, :], lhsT=ident[0:NB, 0:NB],
                                     rhs=gx[:, nsl], start=True, stop=False)
                    pgn.append(pt)

                # parent indices for my 64 nodes (global node ids)
                if l > 0:
                    ppt = rg.tile([NB, 1], i32, tag="ppt", bufs=2, name=f"pp{l}")
                    nc.sync.dma_start(
                        out=ppt[:], in_=pp_my[NB * l : NB * (l + 1), :])
                    G = rg.tile([NB, HCC], f32, tag="G", bufs=2, name=f"G{l}")
                    nc.gpsimd.indirect_dma_start(
                        out=G[:],
                        out_offset=None,
                        in_=state[:, :],
                        in_offset=bass.IndirectOffsetOnAxis(ap=ppt[:, 0:1], axis=0),
                    )
                    h_par = G[:, 0:H]
                    c_par = G[:, H : 2 * H]
                    eidx = rg.tile([NB, 1], f32, tag="eidx", bufs=2, name=f"ei{l}")
                    nc.vector.tensor_scalar_add(eidx[:], G[:, 2 * H : 2 * H + 1], 1.0)
                else:
                    eidx = rg.tile([NB, 1], f32, tag="eidx", bufs=2, name="ei0")
                    nc.vector.memset(eidx[:], 0.0)

                # one-hot(eidx) -> transposed [CL, NB] for use as lhsT
                oh = rg.tile([NB, CL], f32, tag="oh", bufs=2, name=f"oh{l}")
                nc.vector.tensor_scalar(
                    out=oh[:], in0=iotaM[0:NB, :], scalar1=eidx[:, 0:1],
                    scalar2=None, op0=OP.is_equal)
                ohT0 = rg.tile([128, NB], f32, tag="ohT0", bufs=2, name=f"ohT0{l}")
                pt0 = ps.tile([128, NB], f32, tag="ps", name=f"ohtp0{l}")
                nc.tensor.transpose(
                    out=pt0[:], in_=oh[:, 0:128], identity=ident[0:NB, 0:NB])
                nc.scalar.copy(out=ohT0[:], in_=pt0[:])
                ohT1 = rg.tile([CL - 128, NB], f32, tag="ohT1", bufs=2, name=f"ohT1{l}")
                pt1 = ps.tile([CL - 128, NB], f32, tag="ps", name=f"ohtp1{l}")
                nc.tensor.transpose(
                    out=pt1[:], in_=oh[:, 128:CL], identity=ident[0:NB, 0:NB])
                nc.scalar.copy(out=ohT1[:], in_=pt1[:])

                # transpose gathered h_par -> [H, NB] as lhsT blocks
                if l > 0:
                    hpT = rg.tile([128, NB * HT], f32, tag="hpT", bufs=2,
                                  name=f"hpT{l}")
                    for j in range(HT):
                        ptr = ps.tile([128, NB], f32, tag="ps", name=f"hptr{l}_{j}")
                        nc.tensor.transpose(
                            out=ptr[:], in_=h_par[:, 128 * j : 128 * (j + 1)],
                            identity=ident[0:NB, 0:NB])
                        nc.scalar.copy(out=hpT[:, NB * j : NB * (j + 1)], in_=ptr[:])

                # gate matmuls, chunk-major; chunks are [i | f | g | o]
                gact = []  # per-chunk activated tiles
                for n in range(NC_):
                    nsl = slice(512 * n, 512 * (n + 1))
                    pt = pgn[n]
                    nc.tensor.matmul(pt[:, :], lhsT=ohT0[:], rhs=T2b0[:, nsl],
                                     start=False, stop=False)
                    nc.tensor.matmul(pt[:, :], lhsT=ohT1[:], rhs=T2b1[:, nsl],
                                     start=False, stop=(l == 0))
                    if l > 0:
                        for k in range(HT):
                            nc.tensor.matmul(
                                pt[:, :], lhsT=hpT[:, NB * k : NB * (k + 1)],
                                rhs=WhS[k][:, nsl], start=False,
                                stop=(k == HT - 1))
                    act = rg.tile([NB, 512], f32, tag=f"act{n}", bufs=2,
                                  name=f"act{l}_{n}")
                    fn = AF.Tanh if n == 2 else AF.Sigmoid
                    nc.scalar.activation(act[:], pt[:, :], fn)
                    gact.append(act)
                sig_i, sig_f, tanh_g, sig_o = gact

                stage = rg.tile([NB, SC], f32, tag="stage", bufs=2, name=f"st{l}")
                c2 = stage[:, H : 2 * H]
                t2 = rg.tile([NB, H], f32, tag="t2", bufs=2, name=f"t2_{l}")
                nc.vector.tensor_mul(t2[:], sig_i[:], tanh_g[:])
                if l > 0:
                    t1 = rg.tile([NB, H], f32, tag="t1", bufs=2, name=f"t1_{l}")
                    nc.vector.tensor_mul(t1[:], sig_f[:], c_par)
                    nc.vector.tensor_add(c2, t1[:], t2[:])
                else:
                    nc.vector.tensor_copy(c2, t2[:])
                tc2 = rg.tile([NB, H], f32, tag="tc2", bufs=2, name=f"tc2_{l}")
                nc.scalar.activation(tc2[:], c2, AF.Tanh)
                h2 = stage[:, 0:H]
                nc.vector.tensor_mul(h2, sig_o[:], tc2[:])

                # dist = h2 @ oWs + oB   (oWs pre-scaled by dropout)
                h2T = rg.tile([128, NB * HT], f32, tag="h2T", bufs=2, name=f"h2T{l}")
                for j in range(HT):
                    ptr = ps.tile([128, NB], f32, tag="ps", name=f"h2tr{l}_{j}")
                    nc.tensor.transpose(
                        out=ptr[:], in_=stage[:, 128 * j : 128 * (j + 1)],
                        identity=ident[0:NB, 0:NB])
                    nc.scalar.copy(out=h2T[:, NB * j : NB * (j + 1)], in_=ptr[:])
                pd = ps.tile([NB, C], f32, tag="ps", name=f"dist{l}")
                for k in range(HT):
                    nc.tensor.matmul(pd[:, :], lhsT=h2T[:, NB * k : NB * (k + 1)],
                                     rhs=oWs[k][:, :], start=(k == 0), stop=False)
                nc.tensor.matmul(pd[:, :], lhsT=ones[0:1, 0:NB], rhs=oB[0:1, :],
                                 start=False, stop=True)
                nc.vector.tensor_copy(stage[:, HCC:SC], pd[:, :])

                # cm = argmax(dist)
                mx8 = rg.tile([NB, 8], f32, tag="mx8", bufs=2, name=f"mx8_{l}")
                nc.vector.max(mx8[:], stage[:, HCC:SC])
                ix8 = rg.tile([NB, 8], u32, tag="ix8", bufs=2, name=f"ix8_{l}")
                nc.vector.max_index(ix8[:], mx8[:], stage[:, HCC:SC])
                nc.vector.tensor_copy(stage[:, 2 * H : 2 * H + 1], ix8[:, 0:1])

                # publish level rows to every core; ship h/c bytes while the
                # dist/argmax tail still computes
                bounce = dr.tile([NB, SC], f32, tag="bounce", name=f"bn{l}")
                nc.sync.dma_start(out=bounce[:, 0 : 2 * H], in_=stage[:, 0 : 2 * H])
                nc.sync.dma_start(out=bounce[:, 2 * H : SC], in_=stage[:, 2 * H : SC])
                nc.gpsimd.collective_compute(
                    "AllGather",
                    mybir.AluOpType.bypass,
                    replica_groups=[list(range(R))],
                    ins=[bounce[:].opt()],
                    outs=[state[LW * l : LW * (l + 1), :].opt()],
                )

            # ---- schedule ----------------------------------------------------
            for rep in range(reps):
                gx_prep(0)
                gx_prep(1)
                for l in range(DEPTH):
                    if l + 2 < DEPTH:
                        gx_prep(l + 2)
                    recurrence(l)

            # ---- final output gather ----------------------------------------
            for ch in range(FCH):
                osl = slice(FRW * ch, FRW * (ch + 1))
                ot = rg.tile([FRW, 1], i32, tag="ot", bufs=2, name=f"ord{ch}")
                nc.sync.dma_start(out=ot[:], in_=ord_my[osl, :])
                Gf = rg.tile([FRW, 1 + C], f32, tag="Gf", bufs=2, name=f"Gf{ch}")
                nc.gpsimd.indirect_dma_start(
                    out=Gf[:],
                    out_offset=None,
                    in_=state[:, :],
                    in_offset=bass.IndirectOffsetOnAxis(ap=ot[:, 0:1], axis=0),
                    element_offset=2 * H,
                )
                nc.sync.dma_start(out=dist_out[osl, :], in_=Gf[:, 1 : 1 + C])
                cmi = rg.tile([FRW, 1], i32, tag="cmi", bufs=2, name=f"cmi{ch}")
                nc.vector.tensor_copy(cmi[:], Gf[:, 0:1])
                nc.sync.dma_start(out=cm_out[osl, :], in_=cmi[:])

    nc.compile()
    return nc


def get_nc(reps=1):
    key = ("nc", reps)
    if key not in _CACHE:
        _CACHE[key] = _build(reps)
    return _CACHE[key]


def make_in_maps(features, embed_table, Wx, Wh, b, out_W, out_b, dropout_mask,
                 parent_idx, order):
    features = np.asarray(features, np.float32)
    embed_table = np.asarray(embed_table, np.float32)
    Wx = np.asarray(Wx, np.float32)
    Wh = np.asarray(Wh, np.float32)
    b = np.asarray(b, np.float32)
    out_W = np.asarray(out_W, np.float32)
    out_b = np.asarray(out_b, np.float32)
    dropout_mask = np.asarray(dropout_mask, np.float32)
    parent_idx = np.asarray(parent_idx, np.int32)
    order = np.asarray(order, np.int32)

    WhS = np.ascontiguousarray(dropout_mask[:, None] * Wh)
    oWs = np.ascontiguousarray(dropout_mask[:, None] * out_W)
    WxT = np.ascontiguousarray(Wx[:DIN])
    WxB = np.ascontiguousarray(Wx[DIN:])
    embT = np.ascontiguousarray(embed_table.T)
    bRow = b.reshape(1, G4)
    oB = out_b.reshape(1, C)
    iotaM = np.tile(np.arange(CL, dtype=np.float32), (128, 1))
    ident = np.eye(128, dtype=np.float32)
    ones = np.ones((1, 128), np.float32)

    feat_lvl = features.reshape(DEPTH, R, NB, DIN)
    pp_lvl = np.maximum(parent_idx, 0).reshape(DEPTH, R, NB)
    ord_spl = order[: R * OPC].reshape(R, OPC)

    in_maps = []
    for k in range(R):
        in_maps.append({
            "feat_my": np.ascontiguousarray(
                feat_lvl[:, k].reshape(DEPTH * NB, DIN)),
            "pp_my": np.ascontiguousarray(
                pp_lvl[:, k].reshape(DEPTH * NB, 1)),
            "ord_my": np.ascontiguousarray(ord_spl[k].reshape(OPC, 1)),
            "WhS": WhS, "WxT": WxT, "WxB": WxB, "embT": embT,
            "bRow": bRow, "oWs": oWs, "oB": oB,
            "iotaM": iotaM, "ident": ident, "ones": ones,
        })
    return in_maps


def merge_outputs(results):
    dists = np.concatenate([results[k]["dist_out"] for k in range(R)], axis=0)
    cms = np.concatenate([results[k]["cm_out"][:, 0] for k in range(R)], axis=0)
    return dists, cms.astype(np.int32)


def kernel(features, embed_table, Wx, Wh, b, out_W, out_b, dropout_mask,
           parent_idx, order, batch_size=None):
    from concourse import bass_utils

    nc = get_nc()
    in_maps = make_in_maps(features, embed_table, Wx, Wh, b, out_W, out_b,
                           dropout_mask, parent_idx, order)
    res = bass_utils.run_bass_kernel_spmd(nc, in_maps, core_ids=list(range(R)))
    return merge_outputs(res.results)


# revision 10
# speedup vs baseline: 1.0191x; 1.0191x over previous
"""DecoderTreeLSTM Trainium2 kernel (8 NeuronCores, data-parallel over nodes).

Strategy (sharding_hint: data parallel over the 512 trees):
  Each level has 512 nodes; core k owns nodes [64k, 64k+64) of every level.
  Per level, each core computes its 64 nodes' LSTM cell + output head, then an
  AllGather publishes (h, c, dist) rows of the level into a Shared DRAM
  STATE tensor replicated on every core.  Parent lookups (arbitrary row
  indices) are indirect-DMA row gathers from STATE.  The argmax commitment
  is NOT published: consumers recompute it from the gathered parent dist
  rows (identical bytes -> identical argmax), keeping the argmax off the
  publish critical path.

Key algebraic rewrites (all exact):
  - dropout folded into weights: state stores pre-dropout h2;
    Wh and out_W rows are pre-scaled by dropout_mask on the host.
  - embedding contribution: embed_table[eidx] @ Wx_bot + b
      == onehot(eidx) @ (embed_table @ Wx_bot + b[None,:])  (T2b, on-device)
  - features @ Wx_top (72% of FLOPs) has no recurrence dependency; it is
    computed 2 levels ahead so the PE crunches it during AllGather waits.
"""

import sys
import numpy as np

sys.path.insert(0, "/opt/trn_rl_repo")

R = 8          # cores
LW = 512       # nodes per level
NB = LW // R   # 64 nodes per core per level
DEPTH = 16
N = LW * DEPTH
DIN = 512
H = 512
E = 200
C = 150
CL = C + 1     # embedding rows
G4 = 4 * H     # 2048 gate cols ([i | f | g | o])
NC_ = 4        # 512-wide N-chunks of the gate matmul
SC = H + H + C           # 1174 state cols: h, c, dist
DC = H + H               # dist column offset (1024)
OPC = (N - LW) // R  # 960 output rows per core
FCH = 8                  # final-phase chunks
FRW = OPC // FCH         # 120 rows per final chunk

_CACHE = {}


def _build(reps=1):
    import concourse.bass as bass
    import concourse.mybir as mybir
    import concourse.tile as tile
    from concourse import bacc

    f32 = mybir.dt.float32
    i32 = mybir.dt.int32
    u32 = mybir.dt.uint32
    AF = mybir.ActivationFunctionType
    OP = mybir.AluOpType

    nc = bacc.Bacc("TRN2", target_bir_lowering=False, debug=False, num_devices=R)

    # ---- I/O ----------------------------------------------------------------
    feat_my = nc.dram_tensor("feat_my", [DEPTH * NB, DIN], f32, kind="ExternalInput")
    pp_my = nc.dram_tensor("pp_my", [DEPTH * NB, 1], i32, kind="ExternalInput")
    ord_my = nc.dram_tensor("ord_my", [OPC, 1], i32, kind="ExternalInput")
    WhS_in = nc.dram_tensor("WhS", [H, G4], f32, kind="ExternalInput")
    WxT_in = nc.dram_tensor("WxT", [DIN, G4], f32, kind="ExternalInput")
    WxB_in = nc.dram_tensor("WxB", [E, G4], f32, kind="ExternalInput")
    embT_in = nc.dram_tensor("embT", [E, CL], f32, kind="ExternalInput")
    bRow_in = nc.dram_tensor("bRow", [1, G4], f32, kind="ExternalInput")
    oWs_in = nc.dram_tensor("oWs", [H, C], f32, kind="ExternalInput")
    oB_in = nc.dram_tensor("oB", [1, C], f32, kind="ExternalInput")
    iota_in = nc.dram_tensor("iotaM", [128, CL], f32, kind="ExternalInput")
    ident_in = nc.dram_tensor("ident", [128, 128], f32, kind="ExternalInput")
    ones_in = nc.dram_tensor("ones", [1, 128], f32, kind="ExternalInput")

    dist_out = nc.dram_tensor("dist_out", [OPC, C], f32, kind="ExternalOutput")
    cm_out = nc.dram_tensor("cm_out", [OPC, 1], i32, kind="ExternalOutput")

    state = nc.dram_tensor("state", [N, SC], f32, addr_space="Shared")

    HT = H // 128   # 4 hidden 128-tiles

    with tile.TileContext(nc) as tc:
        with (
            tc.tile_pool(name="wpool", bufs=1) as wp,      # resident weights
            tc.tile_pool(name="work", bufs=1) as sb,       # misc single tiles
            tc.tile_pool(name="ring", bufs=1) as rg,       # ring-buffered tiles
            tc.tile_pool(name="pg", bufs=4, space="PSUM") as pg,
            tc.tile_pool(name="px", bufs=1, space="PSUM") as px,
            tc.tile_pool(name="ps", bufs=3, space="PSUM") as ps,
            tc.tile_pool(name="dram", bufs=2, space="DRAM") as dr,
        ):
            # ---- resident constants / weights -------------------------------
            WhS = [wp.tile([128, G4], f32, name=f"WhS{k}") for k in range(HT)]
            for k in range(HT):
                nc.sync.dma_start(out=WhS[k][:], in_=WhS_in[128 * k : 128 * (k + 1), :])
            WxT = [wp.tile([128, G4], f32, name=f"WxT{k}") for k in range(HT)]
            for k in range(HT):
                nc.sync.dma_start(out=WxT[k][:], in_=WxT_in[128 * k : 128 * (k + 1), :])
            WxB0 = wp.tile([128, G4], f32)
            nc.sync.dma_start(out=WxB0[:], in_=WxB_in[0:128, :])
            WxB1 = wp.tile([E - 128, G4], f32)
            nc.sync.dma_start(out=WxB1[:], in_=WxB_in[128:E, :])
            embT0 = wp.tile([128, CL], f32)
            nc.sync.dma_start(out=embT0[:], in_=embT_in[0:128, :])
            embT1 = wp.tile([E - 128, CL], f32)
            nc.sync.dma_start(out=embT1[:], in_=embT_in[128:E, :])
            bRow = wp.tile([1, G4], f32)
            nc.sync.dma_start(out=bRow[:], in_=bRow_in[:, :])
            oWs = [wp.tile([128, C], f32, name=f"oWs{k}") for k in range(HT)]
            for k in range(HT):
                nc.sync.dma_start(out=oWs[k][:], in_=oWs_in[128 * k : 128 * (k + 1), :])
            oB = wp.tile([1, C], f32)
            nc.sync.dma_start(out=oB[:], in_=oB_in[:, :])
            iotaM = wp.tile([128, CL], f32)
            nc.sync.dma_start(out=iotaM[:], in_=iota_in[:, :])
            ident = wp.tile([128, 128], f32)
            nc.sync.dma_start(out=ident[:], in_=ident_in[:, :])
            ones = wp.tile([1, 128], f32)
            nc.sync.dma_start(out=ones[:], in_=ones_in[:, :])

            # ---- T2b = embed_table @ Wx_bot + b  (shape [CL, G4]) -----------
            T2b0 = wp.tile([128, G4], f32)
            T2b1 = wp.tile([CL - 128, G4], f32)
            for (m0, msz, dst) in ((0, 128, T2b0), (128, CL - 128, T2b1)):
                for n in range(NC_):
                    nsl = slice(512 * n, 512 * (n + 1))
                    pt = ps.tile([128, 512], f32, tag="ps", name=f"t2_{m0}_{n}")
                    nc.tensor.matmul(
                        pt[0:msz, :], lhsT=embT0[:, m0 : m0 + msz],
                        rhs=WxB0[:, nsl], start=True, stop=False)
                    nc.tensor.matmul(
                        pt[0:msz, :], lhsT=embT1[:, m0 : m0 + msz],
                        rhs=WxB1[:, nsl], start=False, stop=False)
                    nc.tensor.matmul(
                        pt[0:msz, :], lhsT=ones[0:1, 0:msz],
                        rhs=bRow[0:1, nsl], start=False, stop=True)
                    nc.scalar.copy(out=dst[:, nsl], in_=pt[0:msz, :])

            # ---- per-level pieces -------------------------------------------
            gx_sb = {}

            def gx_prep(l):
                """g_x[l] = features[level l rows] @ Wx_top   (no recurrence dep)."""
                feat = rg.tile([NB, DIN], f32, tag="feat", bufs=3, name=f"feat{l}")
                nc.sync.dma_start(out=feat[:], in_=feat_my[NB * l : NB * (l + 1), :])
                fT = rg.tile([128, NB * HT], f32, tag="fT", bufs=2, name=f"fT{l}")
                for j in range(HT):
                    ptr = ps.tile([128, NB], f32, tag="ps", name=f"ftr{l}_{j}")
                    nc.tensor.transpose(
                        out=ptr[:], in_=feat[:, 128 * j : 128 * (j + 1)],
                        identity=ident[0:NB, 0:NB])
                    nc.scalar.copy(out=fT[:, NB * j : NB * (j + 1)], in_=ptr[:])
                gx = rg.tile([NB, G4], f32, tag="gx", bufs=3, name=f"gx{l}")
                for n in range(NC_):
                    nsl = slice(512 * n, 512 * (n + 1))
                    pxn = px.tile([NB, 512], f32, tag="px", name=f"gxp{l}_{n}")
                    for k in range(HT):
                        nc.tensor.matmul(
                            pxn[:, :], lhsT=fT[:, NB * k : NB * (k + 1)],
                            rhs=WxT[k][:, nsl], start=(k == 0), stop=(k == HT - 1))
                    nc.scalar.copy(out=gx[:, nsl], in_=pxn[:, :])
                gx_sb[l] = gx

            def recurrence(l):
                # Open all four gate-chunk PSUM accumulations with the
                # recurrence-independent g_x term so the PE has ready work
                # while the previous level's AllGather is still in flight.
                gx = gx_sb.pop(l)
                pgn = []
                for n in range(NC_):
                    nsl = slice(512 * n, 512 * (n + 1))
                    pt = pg.tile([NB, 512], f32, tag="pg", name=f"g{l}_{n}")
                    nc.tensor.matmul(pt[:, :], lhsT=ident[0:NB, 0:NB],
                                     rhs=gx[:, nsl], start=True, stop=False)
                    pgn.append(pt)

                # parent indices for my 64 nodes (global node ids)
                if l > 0:
                    ppt = rg.tile([NB, 1], i32, tag="ppt", bufs=2, name=f"pp{l}")
                    nc.sync.dma_start(
                        out=ppt[:], in_=pp_my[NB * l : NB * (l + 1), :])
                    G = rg.tile([NB, SC], f32, tag="G", bufs=2, name=f"G{l}")
                    nc.gpsimd.indirect_dma_start(
                        out=G[:],
                        out_offset=None,
                        in_=state[:, :],
                        in_offset=bass.IndirectOffsetOnAxis(ap=ppt[:, 0:1], axis=0),
                    )
                    h_par = G[:, 0:H]
                    c_par = G[:, H : 2 * H]
                    # parent's committed label = argmax of its gathered dist
                    # (identical bytes -> identical argmax; keeps argmax off
                    # the publish critical path entirely)
                    mx8p = rg.tile([NB, 8], f32, tag="mx8", bufs=2, name=f"mxp{l}")
                    nc.vector.max(mx8p[:], G[:, DC:SC])
                    ix8p = rg.tile([NB, 8], u32, tag="ix8", bufs=2, name=f"ixp{l}")
                    nc.vector.max_index(ix8p[:], mx8p[:], G[:, DC:SC])
                    cmf = rg.tile([NB, 1], f32, tag="eidx", bufs=2, name=f"cmf{l}")
                    nc.vector.tensor_copy(cmf[:], ix8p[:, 0:1])
                else:
                    cmf = rg.tile([NB, 1], f32, tag="eidx", bufs=2, name="cmf0")
                    nc.vector.memset(cmf[:], -1.0)

                # transpose gathered h_par -> [H, NB] as lhsT blocks (emitted
                # first: PE can start these right after the gather while the
                # DVE still derives cm_par/one-hot)
                if l > 0:
                    hpT = rg.tile([128, NB * HT], f32, tag="hpT", bufs=2,
                                  name=f"hpT{l}")
                    for j in range(HT):
                        ptr = ps.tile([128, NB], f32, tag="ps", name=f"hptr{l}_{j}")
                        nc.tensor.transpose(
                            out=ptr[:], in_=h_par[:, 128 * j : 128 * (j + 1)],
                            identity=ident[0:NB, 0:NB])
                        nc.scalar.copy(out=hpT[:, NB * j : NB * (j + 1)], in_=ptr[:])

                # one-hot(cm_par + 1) -> iotaM holds arange(151) - 1, so the
                # is_equal against cm_par lands on column cm_par + 1
                oh = rg.tile([NB, CL], f32, tag="oh", bufs=2, name=f"oh{l}")
                nc.vector.tensor_scalar(
                    out=oh[:], in0=iotaM[0:NB, :], scalar1=cmf[:, 0:1],
                    scalar2=None, op0=OP.is_equal)
                ohT0 = rg.tile([128, NB], f32, tag="ohT0", bufs=2, name=f"ohT0{l}")
                pt0 = ps.tile([128, NB], f32, tag="ps", name=f"ohtp0{l}")
                nc.tensor.transpose(
                    out=pt0[:], in_=oh[:, 0:128], identity=ident[0:NB, 0:NB])
                nc.scalar.copy(out=ohT0[:], in_=pt0[:])
                ohT1 = rg.tile([CL - 128, NB], f32, tag="ohT1", bufs=2, name=f"ohT1{l}")
                pt1 = ps.tile([CL - 128, NB], f32, tag="ps", name=f"ohtp1{l}")
                nc.tensor.transpose(
                    out=pt1[:], in_=oh[:, 128:CL], identity=ident[0:NB, 0:NB])
                nc.scalar.copy(out=ohT1[:], in_=pt1[:])

                # gate matmuls, chunk-major; chunks are [i | f | g | o]
                gact = []  # per-chunk activated tiles
                for n in range(NC_):
                    nsl = slice(512 * n, 512 * (n + 1))
                    pt = pgn[n]
                    if l > 0:
                        for k in range(HT):
                            nc.tensor.matmul(
                                pt[:, :], lhsT=hpT[:, NB * k : NB * (k + 1)],
                                rhs=WhS[k][:, nsl], start=False, stop=False)
                    nc.tensor.matmul(pt[:, :], lhsT=ohT0[:], rhs=T2b0[:, nsl],
                                     start=False, stop=False)
                    nc.tensor.matmul(pt[:, :], lhsT=ohT1[:], rhs=T2b1[:, nsl],
                                     start=False, stop=True)
                    act = rg.tile([NB, 512], f32, tag=f"act{n}", bufs=2,
                                  name=f"act{l}_{n}")
                    fn = AF.Tanh if n == 2 else AF.Sigmoid
                    nc.scalar.activation(act[:], pt[:, :], fn)
                    gact.append(act)
                sig_i, sig_f, tanh_g, sig_o = gact

                stage = rg.tile([NB, SC], f32, tag="stage", bufs=2, name=f"st{l}")
                c2 = stage[:, H : 2 * H]
                t2 = rg.tile([NB, H], f32, tag="t2", bufs=2, name=f"t2_{l}")
                nc.vector.tensor_mul(t2[:], sig_i[:], tanh_g[:])
                if l > 0:
                    t1 = rg.tile([NB, H], f32, tag="t1", bufs=2, name=f"t1_{l}")
                    nc.vector.tensor_mul(t1[:], sig_f[:], c_par)
                    nc.vector.tensor_add(c2, t1[:], t2[:])
                else:
                    nc.vector.tensor_copy(c2, t2[:])
                tc2 = rg.tile([NB, H], f32, tag="tc2", bufs=2, name=f"tc2_{l}")
                nc.scalar.activation(tc2[:], c2, AF.Tanh)
                h2 = stage[:, 0:H]
                nc.vector.tensor_mul(h2, sig_o[:], tc2[:])

                # dist = h2 @ oWs + oB   (oWs pre-scaled by dropout)
                h2T = rg.tile([128, NB * HT], f32, tag="h2T", bufs=2, name=f"h2T{l}")
                for j in range(HT):
                    ptr = ps.tile([128, NB], f32, tag="ps", name=f"h2tr{l}_{j}")
                    nc.tensor.transpose(
                        out=ptr[:], in_=stage[:, 128 * j : 128 * (j + 1)],
                        identity=ident[0:NB, 0:NB])
                    nc.scalar.copy(out=h2T[:, NB * j : NB * (j + 1)], in_=ptr[:])
                pd = ps.tile([NB, C], f32, tag="ps", name=f"dist{l}")
                for k in range(HT):
                    nc.tensor.matmul(pd[:, :], lhsT=h2T[:, NB * k : NB * (k + 1)],
                                     rhs=oWs[k][:, :], start=(k == 0), stop=False)
                nc.tensor.matmul(pd[:, :], lhsT=ones[0:1, 0:NB], rhs=oB[0:1, :],
                                 start=False, stop=True)
                nc.vector.tensor_copy(stage[:, DC:SC], pd[:, :])

                # publish level rows to every core; ship h/c bytes while the
                # dist/argmax tail still computes
                bounce = dr.tile([NB, SC], f32, tag="bounce", name=f"bn{l}")
                nc.sync.dma_start(out=bounce[:, 0:DC], in_=stage[:, 0:DC])
                nc.sync.dma_start(out=bounce[:, DC:SC], in_=stage[:, DC:SC])
                nc.gpsimd.collective_compute(
                    "AllGather",
                    mybir.AluOpType.bypass,
                    replica_groups=[list(range(R))],
                    ins=[bounce[:].opt()],
                    outs=[state[LW * l : LW * (l + 1), :].opt()],
                )

            # ---- schedule ----------------------------------------------------
            for rep in range(reps):
                gx_prep(0)
                gx_prep(1)
                for l in range(DEPTH):
                    if l + 2 < DEPTH:
                        gx_prep(l + 2)
                    recurrence(l)

            # ---- final output gather ----------------------------------------
            for ch in range(FCH):
                osl = slice(FRW * ch, FRW * (ch + 1))
                ot = rg.tile([FRW, 1], i32, tag="ot", bufs=2, name=f"ord{ch}")
                nc.sync.dma_start(out=ot[:], in_=ord_my[osl, :])
                Gf = rg.tile([FRW, C], f32, tag="Gf", bufs=2, name=f"Gf{ch}")
                nc.gpsimd.indirect_dma_start(
                    out=Gf[:],
                    out_offset=None,
                    in_=state[:, :],
                    in_offset=bass.IndirectOffsetOnAxis(ap=ot[:, 0:1], axis=0),
                    element_offset=DC,
                )
                nc.sync.dma_start(out=dist_out[osl, :], in_=Gf[:, :])
                mx8f = rg.tile([FRW, 8], f32, tag="mx8f", bufs=2, name=f"mxf{ch}")
                nc.vector.max(mx8f[:], Gf[:, :])
                ix8f = rg.tile([FRW, 8], u32, tag="ix8f", bufs=2, name=f"ixf{ch}")
                nc.vector.max_index(ix8f[:], mx8f[:], Gf[:, :])
                cmi = rg.tile([FRW, 1], i32, tag="cmi", bufs=2, name=f"cmi{ch}")
                nc.vector.tensor_copy(cmi[:], ix8f[:, 0:1])
                nc.sync.dma_start(out=cm_out[osl, :], in_=cmi[:])

    nc.compile()
    return nc


def get_nc(reps=1):
    key = ("nc", reps)
    if key not in _CACHE:
        _CACHE[key] = _build(reps)
    return _CACHE[key]


def make_in_maps(features, embed_table, Wx, Wh, b, out_W, out_b, dropout_mask,
                 parent_idx, order):
    features = np.asarray(features, np.float32)
    embed_table = np.asarray(embed_table, np.float32)
    Wx = np.asarray(Wx, np.float32)
    Wh = np.asarray(Wh, np.float32)
    b = np.asarray(b, np.float32)
    out_W = np.asarray(out_W, np.float32)
    out_b = np.asarray(out_b, np.float32)
    dropout_mask = np.asarray(dropout_mask, np.float32)
    parent_idx = np.asarray(parent_idx, np.int32)
    order = np.asarray(order, np.int32)

    WhS = np.ascontiguousarray(dropout_mask[:, None] * Wh)
    oWs = np.ascontiguousarray(dropout_mask[:, None] * out_W)
    WxT = np.ascontiguousarray(Wx[:DIN])
    WxB = np.ascontiguousarray(Wx[DIN:])
    embT = np.ascontiguousarray(embed_table.T)
    bRow = b.reshape(1, G4)
    oB = out_b.reshape(1, C)
    iotaM = np.tile(np.arange(CL, dtype=np.float32) - 1.0, (128, 1))
    ident = np.eye(128, dtype=np.float32)
    ones = np.ones((1, 128), np.float32)

    feat_lvl = features.reshape(DEPTH, R, NB, DIN)
    pp_lvl = np.maximum(parent_idx, 0).reshape(DEPTH, R, NB)
    ord_spl = order[: R * OPC].reshape(R, OPC)

    in_maps = []
    for k in range(R):
        in_maps.append({
            "feat_my": np.ascontiguousarray(
                feat_lvl[:, k].reshape(DEPTH * NB, DIN)),
            "pp_my": np.ascontiguousarray(
                pp_lvl[:, k].reshape(DEPTH * NB, 1)),
            "ord_my": np.ascontiguousarray(ord_spl[k].reshape(OPC, 1)),
            "WhS": WhS, "WxT": WxT, "WxB": WxB, "embT": embT,
            "bRow": bRow, "oWs": oWs, "oB": oB,
            "iotaM": iotaM, "ident": ident, "ones": ones,
        })
    return in_maps


def merge_outputs(results):
    dists = np.concatenate([results[k]["dist_out"] for k in range(R)], axis=0)
    cms = np.concatenate([results[k]["cm_out"][:, 0] for k in range(R)], axis=0)
    return dists, cms.astype(np.int32)


def kernel(features, embed_table, Wx, Wh, b, out_W, out_b, dropout_mask,
           parent_idx, order, batch_size=None):
    from concourse import bass_utils

    nc = get_nc()
    in_maps = make_in_maps(features, embed_table, Wx, Wh, b, out_W, out_b,
                           dropout_mask, parent_idx, order)
    res = bass_utils.run_bass_kernel_spmd(nc, in_maps, core_ids=list(range(R)))
    return merge_outputs(res.results)
